# revision 26
# baseline (speedup 1.0000x reference)
"""Bass/Trainium2 kernel for nn_BiGRIL (gnn_message_passing).

Algebra (h == 0, C == 1 make the network collapse):
  x1  = where(mask, x, b_fs)
  v   = PA^T . streams            streams = [x1, m, xg, mg, cg, 1]
        with xg = A^T x1, mg = A^T m, cg = A^T 1   (graph diffusion)
  o   = PReLU(v) = max(a*v, v)    (ACT Prelu, one op per 4 chunks)
  f   = PB^T o                    PB = outer(w_ro1, W_o1[:,0])  (rank-1)
  rr  = relu(f + bk)              bk = W_o1[:,0]*b_ro + b_o1    (DVE, one op)
  out = W_o2 . rr  (+ b_o2 added on host)

Layout: t-major columns, col = t*N + n.  Sharding: data-parallel over
batch (B=8 -> 8 cores), weights + adj replicated, no collectives.

PE mapping: every matmul is full-array K=128/M=128/N=512 (no array
tiling, no mode switches -> the HAM clock gate stays at 8/8 = 2.4 GHz).
Each pass packs TWO 512-col chunks block-diagonally into K:
  A: lhsT rows 0:6  -> cols 0:64  (even chunk streams -> v)
     lhsT rows 64:70-> cols 64:128 (odd chunk streams -> v)
  B: [PB 0; 0 PB], C: col 2j+s = W_o2 on rows 64s:64s+64
so each of A/B/C streams only NT/2 columns.  C accumulates 32 pairs
(64 chunks) of output rows into one PSUM bank via column-shifted
stationaries (start=False); 2 evacuations per kernel.
G-phase keeps adj as the *moving* operand (8 LDWEIGHTS total).
"""

import numpy as np
import sys

sys.path.insert(0, "/opt/trn_rl_repo")

B, C, N, T = 8, 1, 1024, 64
H = 64
NT = N * T            # 65536 columns per core, col = t*N + n
CHUNK = 512
NPAIR = 64            # pair p = t-step p: chunks (2p, 2p+1) = n-halves
NGRP = 32             # grp g = pairs (2g, 2g+1) -> one [128,1024] X tile
BLK = 2048            # ma-tile columns = 4 pairs (4 t-steps)
NBLK = 16
NSC = 32              # column-shifted W_o2 variants (pairs per out bank)

_CACHE = {}


def _fold_weights(W_fs, b_fs, W_in, b_in, W_gc, b_gc, W_lo, b_lo, prelu_a,
                  W_ro, b_ro, W_o1, b_o1, W_o2, b_o2, adj):
    f8 = np.float64
    W_in, b_in = W_in.astype(f8), b_in.astype(f8)
    W_gc, b_gc = W_gc.astype(f8), b_gc.astype(f8)
    W_lo, b_lo = W_lo.astype(f8), b_lo.astype(f8)
    W_ro, b_ro = W_ro.astype(f8), b_ro.astype(f8)
    W_o1, b_o1 = W_o1.astype(f8), b_o1.astype(f8)
    W_o2, b_o2 = W_o2.astype(f8), b_o2.astype(f8)

    W0 = W_in[:, 0]
    W1 = W_in[:, 1]
    Wlo1 = W_lo[:, :H]
    M1 = Wlo1 @ W_gc[:, :H]
    M2 = Wlo1 @ W_gc[:, H:]
    b_fold = Wlo1 @ b_gc + b_lo

    PA6 = np.stack([
        M1 @ W0, M1 @ W1, M2 @ W0, M2 @ W1,
        M2 @ b_in, M1 @ b_in + b_fold,
    ])                                      # [6, 64]

    w_ro1 = W_ro[0, :H]
    W_o1c = W_o1[:, 0]
    PB = np.outer(w_ro1, W_o1c)             # [64(K=o), 64(M=f)]
    bk = W_o1c * b_ro[0] + b_o1             # [64]

    h16, fp = np.float16, np.float32
    sa = np.zeros((128, 128))
    sa[0:6, 0:64] = PA6
    sa[64:70, 64:128] = PA6
    sb = np.zeros((128, 128))
    sb[0:64, 0:64] = PB
    sb[64:128, 64:128] = PB
    sc = np.zeros((128, NSC * 128))
    for j in range(NSC):
        sc[0:64, 128 * j + 2 * j] = W_o2[0]
        sc[64:128, 128 * j + 2 * j + 1] = W_o2[0]
    bk2 = np.zeros((128, 1))
    bk2[0:64, 0] = bk
    bk2[64:128, 0] = bk

    cg = adj.astype(f8).sum(axis=0)         # [N] col sums of adj
    cge = np.tile(cg[:512], 4)              # even-chunk cg row per ma block
    cgo = np.tile(cg[512:], 4)

    return dict(
        sa=sa.astype(h16), sb=sb.astype(h16), sc=sc.astype(h16),
        bk2=bk2.astype(fp), cge=cge.astype(h16), cgo=cgo.astype(h16),
        ones=np.ones(BLK, h16),
        bfs=float(b_fs[0]), a=float(prelu_a), b_o2=float(b_o2[0]),
    )


def _build_program(a_slope, bfs_val):
    import concourse.bass as bass
    import concourse.bacc as bacc
    import concourse.mybir as mybir
    import concourse.tile as tile

    dt = mybir.dt
    f32 = dt.float32
    h16 = dt.float16
    AF = mybir.ActivationFunctionType
    ALU = mybir.AluOpType

    nc = bacc.Bacc("TRN2", target_bir_lowering=False, debug=False,
                   num_devices=B)

    xT_d = nc.dram_tensor("xT", [T, N], h16, kind="ExternalInput")
    mT_d = nc.dram_tensor("mT", [T, N], h16, kind="ExternalInput")
    xn_d = nc.dram_tensor("xn", [128, 512], h16, kind="ExternalInput")
    mn_d = nc.dram_tensor("mn", [128, 512], h16, kind="ExternalInput")
    adj_d = nc.dram_tensor("adj", [N, N], h16, kind="ExternalInput")
    cge_d = nc.dram_tensor("cge", [BLK], h16, kind="ExternalInput")
    ones_d = nc.dram_tensor("ones", [BLK], h16, kind="ExternalInput")
    cgo_d = nc.dram_tensor("cgo", [BLK], h16, kind="ExternalInput")
    sa_d = nc.dram_tensor("sa", [128, 128], h16, kind="ExternalInput")
    sb_d = nc.dram_tensor("sb", [128, 128], h16, kind="ExternalInput")
    sc_d = nc.dram_tensor("sc", [128, NSC * 128], h16, kind="ExternalInput")
    bk2_d = nc.dram_tensor("bk2", [128, 1], f32, kind="ExternalInput")
    out_d = nc.dram_tensor("out", [2 * NPAIR, CHUNK], f32,
                           kind="ExternalOutput")

    from contextlib import ExitStack
    with tile.TileContext(nc) as tc, ExitStack() as ctx:
        const = ctx.enter_context(tc.tile_pool(name="const", bufs=1))
        adjp = ctx.enter_context(tc.tile_pool(name="adjp", bufs=1))
        movap = ctx.enter_context(tc.tile_pool(name="movap", bufs=1))
        ttp = ctx.enter_context(tc.tile_pool(name="ttp", bufs=4))
        rtp = ctx.enter_context(tc.tile_pool(name="rtp", bufs=4))
        osp = ctx.enter_context(tc.tile_pool(name="osp", bufs=2))
        Xp = ctx.enter_context(tc.tile_pool(name="Xp", bufs=3, space="PSUM"))
        gp = ctx.enter_context(tc.tile_pool(name="gp", bufs=2, space="PSUM"))

        sa_t = const.tile([128, 128], h16)
        sb_t = const.tile([128, 128], h16)
        sc_t = const.tile([128, NSC * 128], h16)
        bk2_t = const.tile([128, 1], f32)
        nc.sync.dma_start(out=sa_t[:], in_=sa_d[:])
        nc.sync.dma_start(out=sb_t[:], in_=sb_d[:])
        nc.sync.dma_start(out=sc_t[:], in_=sc_d[:])
        nc.sync.dma_start(out=bk2_t[:], in_=bk2_d[:])

        xT_t = const.tile([T, N], h16)
        mT_t = const.tile([T, N], h16)
        x1T_t = const.tile([T, N], h16)
        xn_t = const.tile([128, 512], h16)
        mn_t = const.tile([128, 512], h16)
        x1n_t = const.tile([128, 512], h16)
        gall = const.tile([128, 1024], h16)
        gxT = const.tile([128, 1024], h16)
        nc.scalar.dma_start(out=xT_t[:], in_=xT_d[:])
        nc.scalar.dma_start(out=mT_t[:], in_=mT_d[:])
        nc.scalar.dma_start(out=xn_t[:], in_=xn_d[:])
        nc.scalar.dma_start(out=mn_t[:], in_=mn_d[:])

        # x1 = (x - bfs)*m + bfs  in both orientations
        nc.vector.scalar_tensor_tensor(
            out=x1T_t[:], in0=xT_t[:], scalar=bfs_val, in1=mT_t[:],
            op0=ALU.subtract, op1=ALU.mult)
        nc.vector.tensor_scalar_add(x1T_t[:], x1T_t[:], bfs_val)
        nc.vector.scalar_tensor_tensor(
            out=x1n_t[:], in0=xn_t[:], scalar=bfs_val, in1=mn_t[:],
            op0=ALU.subtract, op1=ALU.mult)
        nc.vector.tensor_scalar_add(x1n_t[:], x1n_t[:], bfs_val)

        # gall: G stationary [128 n-parts, (x1 64t | m 64t)] per node-group
        for nt in range(8):
            nc.scalar.dma_start(out=gall[:, 128 * nt:128 * nt + 64],
                                in_=x1n_t[:, 64 * nt:64 * nt + 64])
            nc.gpsimd.dma_start(out=gall[:, 128 * nt + 64:128 * nt + 128],
                                in_=mn_t[:, 64 * nt:64 * nt + 64])

        adjt = []
        for nt in range(8):
            at = adjp.tile([128, 1024], h16, tag=f"adjt{nt}",
                           name=f"adjt{nt}")
            nc.sync.dma_start(out=at[:], in_=adj_d[nt * 128:(nt + 1) * 128, :])
            adjt.append(at)

        # ---- G: Xg[t | 64+t, m] = sum_n [x1|m][n,t] * adj[n,m] ----------
        Xg = Xp.tile([128, 1024], f32, tag="X", name="Xg")
        for nt in range(8):
            for hf in range(2):
                nc.tensor.matmul(
                    Xg[:, 512 * hf:512 * hf + 512],
                    gall[:, 128 * nt:128 * nt + 128],
                    adjt[nt][:, 512 * hf:512 * hf + 512],
                    start=(nt == 0), stop=(nt == 7))
        nc.vector.tensor_copy(gxT[:, 0:512], Xg[:, 0:512])
        nc.scalar.copy(gxT[:, 512:1024], Xg[:, 512:1024])

        # ---- persistent ma tiles: [128, 2048] = 4 pairs (t-steps) -------
        # rows 0:6  = even-chunk streams (n 0:512):  x1, m, xg, mg, cg, 1
        # rows 64:70= odd-chunk streams  (n 512:1024)
        ma4 = []
        for i in range(4):
            mai = movap.tile([128, BLK], h16, tag=f"ma{i}", name=f"ma{i}")
            nc.vector.memset(mai[0:64, :], 0.0)
            nc.vector.memset(mai[64:128, :], 0.0)
            nc.sync.dma_start(out=mai[5:6, :], in_=ones_d[:])
            nc.sync.dma_start(out=mai[69:70, :], in_=ones_d[:])
            nc.sync.dma_start(out=mai[4:5, :], in_=cge_d[:])
            nc.sync.dma_start(out=mai[68:69, :], in_=cgo_d[:])
            ma4.append(mai)

        def emit_ma(b):
            mat = ma4[b % 4]
            t0 = 4 * b
            nc.sync.dma_start(out=mat[0:1, :], in_=x1T_t[t0:t0 + 4, 0:512])
            nc.sync.dma_start(out=mat[64:65, :], in_=x1T_t[t0:t0 + 4, 512:1024])
            nc.scalar.dma_start(out=mat[1:2, :], in_=mT_t[t0:t0 + 4, 0:512])
            nc.scalar.dma_start(out=mat[65:66, :],
                                in_=mT_t[t0:t0 + 4, 512:1024])
            nc.gpsimd.dma_start(out=mat[2:3, :], in_=gxT[t0:t0 + 4, 0:512])
            nc.sync.dma_start(out=mat[66:67, :], in_=gxT[t0:t0 + 4, 512:1024])
            nc.gpsimd.dma_start(out=mat[3:4, :],
                                in_=gxT[64 + t0:64 + t0 + 4, 0:512])
            nc.scalar.dma_start(out=mat[67:68, :],
                                in_=gxT[64 + t0:64 + t0 + 4, 512:1024])

        emit_ma(0)
        emit_ma(1)

        # ---- streaming pipeline -----------------------------------------
        Xs = {}
        tts = {}
        tt_all = {}
        rts = {}
        gcur = {}

        def pe_dep(ap):
            # dependency-carrying PE no-op: pins PE program order to the
            # producer of `ap` so the scheduler cannot hoist the next MMs
            bi = nc.tensor.nop(hint="dep")
            bi.ins.ins = [nc.tensor.lower_ap(ap)]

        def st_A(g):
            X = Xp.tile([128, 1024], f32, tag="X", name=f"X{g}")
            for h in range(2):
                p = 2 * g + h
                mat = ma4[(p // 4) % 4]
                mc = (p % 4) * CHUNK
                nc.tensor.matmul(
                    X[:, 512 * h:512 * h + 512], sa_t[:],
                    mat[:, mc:mc + CHUNK], start=True, stop=True)
            Xs[g] = X

        def st_t(g):
            tt = ttp.tile([128, 1024], h16, tag="tt", name=f"tt{g}")
            nc.scalar.activation(tt[:], Xs[g][:], AF.Prelu,
                                 bias=0.0, scale=1.0, alpha=a_slope)
            tts[g] = tt
            tt_all[g] = tt

        def st_B(g):
            X, tt = Xs[g], tts[g]
            for h in range(2):
                nc.tensor.matmul(
                    X[:, 512 * h:512 * h + 512], sb_t[:],
                    tt[:, 512 * h:512 * h + 512], start=True, stop=True)
            del tts[g]

        def st_rr(g):
            rt = rtp.tile([128, 1024], h16, tag="rt", name=f"rt{g}")
            X = Xs[g]
            nc.vector.tensor_scalar(
                out=rt[:, 0:512], in0=X[:, 0:512],
                scalar1=bk2_t[:, 0:1], scalar2=0.0,
                op0=ALU.add, op1=ALU.max)
            nc.scalar.activation(rt[:, 512:1024], X[:, 512:1024], AF.Relu,
                                 bias=bk2_t[:, 0:1], scale=1.0)
            rts[g] = rt
            del Xs[g]

        def st_C(g):
            rt = rts[g]
            for h in range(2):
                p = 2 * g + h
                s = p // NSC
                j = p % NSC
                if j == 0:
                    gcur[s] = gp.tile([128, 512], f32, tag="go",
                                      name=f"go{s}")
                gam = gcur[s]
                nc.tensor.matmul(
                    gam[:], sc_t[:, 128 * j:128 * j + 128],
                    rt[:, 512 * h:512 * h + 512],
                    start=(j == 0), stop=(j == NSC - 1),
                    skip_group_check=True)
                if j == NSC - 1:
                    osb = osp.tile([128, 512], f32, tag="os", name=f"os{s}")
                    nc.scalar.copy(osb[0:64, :], gam[0:64, :])
                    nc.sync.dma_start(out=out_d[64 * s:64 * s + 64, :],
                                      in_=osb[0:64, :])
                    del gcur[s]
            del rts[g]

        # lags: B at g-2, C at g-4 -> every PE dep has ~2 iterations of
        # slack against the ACT/DVE elementwise ops.
        for g in range(NGRP + 4):
            if 2 <= g < NGRP + 2:
                st_B(g - 2)
                st_rr(g - 2)
            if g >= 4:
                st_C(g - 4)
            if g < NGRP:
                if g % 2 == 0 and g // 2 + 2 < NBLK:
                    emit_ma(g // 2 + 2)
                st_A(g)
                st_t(g)
    nc.compile()
    return nc


def _get_program(a_slope, bfs_val):
    if "prog" not in _CACHE:
        _CACHE["prog"] = _build_program(a_slope, bfs_val)
    return _CACHE["prog"]


def _make_in_maps(x, mask_f, adj16, folded):
    shared = dict(adj=adj16, cge=folded["cge"], cgo=folded["cgo"],
                  ones=folded["ones"],
                  sa=folded["sa"], sb=folded["sb"], sc=folded["sc"],
                  bk2=folded["bk2"])
    in_maps = []
    for b in range(B):
        xb = np.ascontiguousarray(x[b, 0]).astype(np.float16)   # [N, T]
        mb = np.ascontiguousarray(mask_f[b, 0])                 # [N, T] f16
        m = dict(shared)
        m["xT"] = np.ascontiguousarray(xb.T)
        m["mT"] = np.ascontiguousarray(mb.T)
        m["xn"] = np.ascontiguousarray(
            xb.reshape(8, 128, T).transpose(1, 0, 2).reshape(128, 512))
        m["mn"] = np.ascontiguousarray(
            mb.reshape(8, 128, T).transpose(1, 0, 2).reshape(128, 512))
        in_maps.append(m)
    return in_maps


def kernel(x, mask, W_fs, b_fs, W_in, b_in, adj, W_gc, b_gc, W_lo, b_lo,
           prelu_a, W_ro, b_ro, W_o1, b_o1, W_o2, b_o2):
    x = np.asarray(x, np.float32)
    mask_f = np.asarray(mask, np.float16)
    adj = np.asarray(adj, np.float32)

    folded = _fold_weights(np.asarray(W_fs), np.asarray(b_fs),
                           np.asarray(W_in), np.asarray(b_in),
                           np.asarray(W_gc), np.asarray(b_gc),
                           np.asarray(W_lo), np.asarray(b_lo),
                           float(prelu_a),
                           np.asarray(W_ro), np.asarray(b_ro),
                           np.asarray(W_o1), np.asarray(b_o1),
                           np.asarray(W_o2), np.asarray(b_o2), adj)

    nc = _get_program(folded["a"], folded["bfs"])
    in_maps = _make_in_maps(x, mask_f, adj.astype(np.float16), folded)

    from concourse.bass_utils import run_bass_kernel_spmd
    res = run_bass_kernel_spmd(nc, in_maps, list(range(B)))

    out = np.empty((B, C, N, T), np.float32)
    for b in range(B):
        # out row r = chunk r; chunk 2p+h = (t=p, n-half h)
        ob = np.asarray(res.results[b]["out"]).reshape(T, 2, 512)
        ob = ob.transpose(0, 1, 2).reshape(T, N)   # [t, n]
        out[b, 0] = ob.T + folded["b_o2"]
    return out


# revision 27
# speedup vs baseline: 1.0518x; 1.0518x over previous
"""Bass/Trainium2 kernel for nn_BiGRIL (gnn_message_passing).

Algebra (h == 0, C == 1 make the network collapse):
  x1  = where(mask, x, b_fs)
  v   = PA^T . streams            streams = [x1, m, xg, mg, cg, 1]
        with xg = A^T x1, mg = A^T m, cg = A^T 1   (graph diffusion)
  o   = PReLU(v) = max(a*v, v)    (ACT Prelu, one op per 4 chunks)
  f   = PB^T o                    PB = outer(w_ro1, W_o1[:,0])  (rank-1)
  rr  = relu(f + bk)              bk = W_o1[:,0]*b_ro + b_o1    (DVE, one op)
  out = W_o2 . rr  (+ b_o2 added on host)

Layout: t-major columns, col = t*N + n.  Sharding: data-parallel over
batch (B=8 -> 8 cores), weights + adj replicated, no collectives.

PE mapping: every matmul is full-array K=128/M=128/N=512 (no array
tiling, no mode switches -> the HAM clock gate stays at 8/8 = 2.4 GHz).
Each pass packs TWO 512-col chunks block-diagonally into K:
  A: lhsT rows 0:6  -> cols 0:64  (even chunk streams -> v)
     lhsT rows 64:70-> cols 64:128 (odd chunk streams -> v)
  B: [PB 0; 0 PB], C: col 2j+s = W_o2 on rows 64s:64s+64
so each of A/B/C streams only NT/2 columns.  C accumulates 32 pairs
(64 chunks) of output rows into one PSUM bank via column-shifted
stationaries (start=False); 2 evacuations per kernel.
G-phase keeps adj as the *moving* operand (8 LDWEIGHTS total).
"""

import numpy as np
import sys

sys.path.insert(0, "/opt/trn_rl_repo")

B, C, N, T = 8, 1, 1024, 64
H = 64
NT = N * T            # 65536 columns per core, col = t*N + n
CHUNK = 512
NPAIR = 64            # pair p = t-step p: chunks (2p, 2p+1) = n-halves
NGRP = 32             # grp g = pairs (2g, 2g+1) -> one [128,1024] X tile
BLK = 2048            # ma-tile columns = 4 pairs (4 t-steps)
NBLK = 16
NSC = 32              # column-shifted W_o2 variants (pairs per out bank)

_CACHE = {}


def _fold_weights(W_fs, b_fs, W_in, b_in, W_gc, b_gc, W_lo, b_lo, prelu_a,
                  W_ro, b_ro, W_o1, b_o1, W_o2, b_o2, adj):
    f8 = np.float64
    W_in, b_in = W_in.astype(f8), b_in.astype(f8)
    W_gc, b_gc = W_gc.astype(f8), b_gc.astype(f8)
    W_lo, b_lo = W_lo.astype(f8), b_lo.astype(f8)
    W_ro, b_ro = W_ro.astype(f8), b_ro.astype(f8)
    W_o1, b_o1 = W_o1.astype(f8), b_o1.astype(f8)
    W_o2, b_o2 = W_o2.astype(f8), b_o2.astype(f8)

    W0 = W_in[:, 0]
    W1 = W_in[:, 1]
    Wlo1 = W_lo[:, :H]
    M1 = Wlo1 @ W_gc[:, :H]
    M2 = Wlo1 @ W_gc[:, H:]
    b_fold = Wlo1 @ b_gc + b_lo

    PA6 = np.stack([
        M1 @ W0, M1 @ W1, M2 @ W0, M2 @ W1,
        M2 @ b_in, M1 @ b_in + b_fold,
    ])                                      # [6, 64]

    w_ro1 = W_ro[0, :H]
    W_o1c = W_o1[:, 0]
    PB = np.outer(w_ro1, W_o1c)             # [64(K=o), 64(M=f)]
    bk = W_o1c * b_ro[0] + b_o1             # [64]

    h16, fp = np.float16, np.float32
    sa = np.zeros((128, 128))
    sa[0:6, 0:64] = PA6
    sa[64:70, 64:128] = PA6
    sb = np.zeros((128, 128))
    sb[0:64, 0:64] = PB
    sb[64:128, 64:128] = PB
    sc = np.zeros((128, NSC * 128))
    for j in range(NSC):
        sc[0:64, 128 * j + 2 * j] = W_o2[0]
        sc[64:128, 128 * j + 2 * j + 1] = W_o2[0]
    bk2 = np.zeros((128, 1))
    bk2[0:64, 0] = bk
    bk2[64:128, 0] = bk

    cg = adj.astype(f8).sum(axis=0)         # [N] col sums of adj
    cge = np.tile(cg[:512], 4)              # even-chunk cg row per ma block
    cgo = np.tile(cg[512:], 4)

    return dict(
        sa=sa.astype(h16), sb=sb.astype(h16), sc=sc.astype(h16),
        bk2=bk2.astype(fp), cge=cge.astype(h16), cgo=cgo.astype(h16),
        ones=np.ones(BLK, h16),
        bfs=float(b_fs[0]), a=float(prelu_a), b_o2=float(b_o2[0]),
    )


def _build_program(a_slope, bfs_val):
    import concourse.bass as bass
    import concourse.bacc as bacc
    import concourse.mybir as mybir
    import concourse.tile as tile

    dt = mybir.dt
    f32 = dt.float32
    h16 = dt.float16
    AF = mybir.ActivationFunctionType
    ALU = mybir.AluOpType

    nc = bacc.Bacc("TRN2", target_bir_lowering=False, debug=False,
                   num_devices=B)

    xT_d = nc.dram_tensor("xT", [T, N], h16, kind="ExternalInput")
    mT_d = nc.dram_tensor("mT", [T, N], h16, kind="ExternalInput")
    xn_d = nc.dram_tensor("xn", [128, 512], h16, kind="ExternalInput")
    mn_d = nc.dram_tensor("mn", [128, 512], h16, kind="ExternalInput")
    adj_d = nc.dram_tensor("adj", [N, N], h16, kind="ExternalInput")
    cge_d = nc.dram_tensor("cge", [BLK], h16, kind="ExternalInput")
    ones_d = nc.dram_tensor("ones", [BLK], h16, kind="ExternalInput")
    cgo_d = nc.dram_tensor("cgo", [BLK], h16, kind="ExternalInput")
    sa_d = nc.dram_tensor("sa", [128, 128], h16, kind="ExternalInput")
    sb_d = nc.dram_tensor("sb", [128, 128], h16, kind="ExternalInput")
    sc_d = nc.dram_tensor("sc", [128, NSC * 128], h16, kind="ExternalInput")
    bk2_d = nc.dram_tensor("bk2", [128, 1], f32, kind="ExternalInput")
    out_d = nc.dram_tensor("out", [2 * NPAIR, CHUNK], f32,
                           kind="ExternalOutput")

    from contextlib import ExitStack
    with tile.TileContext(nc) as tc, ExitStack() as ctx:
        const = ctx.enter_context(tc.tile_pool(name="const", bufs=1))
        adjp = ctx.enter_context(tc.tile_pool(name="adjp", bufs=1))
        movap = ctx.enter_context(tc.tile_pool(name="movap", bufs=1))
        ttp = ctx.enter_context(tc.tile_pool(name="ttp", bufs=4))
        rtp = ctx.enter_context(tc.tile_pool(name="rtp", bufs=4))
        osp = ctx.enter_context(tc.tile_pool(name="osp", bufs=2))
        Xp = ctx.enter_context(tc.tile_pool(name="Xp", bufs=3, space="PSUM"))
        gp = ctx.enter_context(tc.tile_pool(name="gp", bufs=2, space="PSUM"))

        sa_t = const.tile([128, 128], h16)
        sb_t = const.tile([128, 128], h16)
        sc_t = const.tile([128, NSC * 128], h16)
        bk2_t = const.tile([128, 1], f32)
        nc.sync.dma_start(out=sa_t[:], in_=sa_d[:])
        nc.sync.dma_start(out=sb_t[:], in_=sb_d[:])
        nc.sync.dma_start(out=sc_t[:], in_=sc_d[:])
        nc.sync.dma_start(out=bk2_t[:], in_=bk2_d[:])

        xT_t = const.tile([T, N], h16)
        mT_t = const.tile([T, N], h16)
        x1T_t = const.tile([T, N], h16)
        xn_t = const.tile([128, 512], h16)
        mn_t = const.tile([128, 512], h16)
        x1n_t = const.tile([128, 512], h16)
        gall = const.tile([128, 1024], h16)
        gxT = const.tile([128, 1024], h16)
        nc.scalar.dma_start(out=xT_t[:], in_=xT_d[:])
        nc.scalar.dma_start(out=mT_t[:], in_=mT_d[:])
        nc.scalar.dma_start(out=xn_t[:], in_=xn_d[:])
        nc.scalar.dma_start(out=mn_t[:], in_=mn_d[:])

        # x1 = (x - bfs)*m + bfs  in both orientations
        nc.vector.scalar_tensor_tensor(
            out=x1T_t[:], in0=xT_t[:], scalar=bfs_val, in1=mT_t[:],
            op0=ALU.subtract, op1=ALU.mult)
        nc.vector.tensor_scalar_add(x1T_t[:], x1T_t[:], bfs_val)
        nc.vector.scalar_tensor_tensor(
            out=x1n_t[:], in0=xn_t[:], scalar=bfs_val, in1=mn_t[:],
            op0=ALU.subtract, op1=ALU.mult)
        nc.vector.tensor_scalar_add(x1n_t[:], x1n_t[:], bfs_val)

        # gall: G stationary [128 n-parts, (x1 64t | m 64t)] per node-group
        for nt in range(8):
            nc.scalar.dma_start(out=gall[:, 128 * nt:128 * nt + 64],
                                in_=x1n_t[:, 64 * nt:64 * nt + 64])
            nc.gpsimd.dma_start(out=gall[:, 128 * nt + 64:128 * nt + 128],
                                in_=mn_t[:, 64 * nt:64 * nt + 64])

        adjt = []
        for nt in range(8):
            at = adjp.tile([128, 1024], h16, tag=f"adjt{nt}",
                           name=f"adjt{nt}")
            nc.sync.dma_start(out=at[:], in_=adj_d[nt * 128:(nt + 1) * 128, :])
            adjt.append(at)

        # ---- G: Xg[t | 64+t, m] = sum_n [x1|m][n,t] * adj[n,m] ----------
        Xg = Xp.tile([128, 1024], f32, tag="X", name="Xg")
        for nt in range(8):
            for hf in range(2):
                nc.tensor.matmul(
                    Xg[:, 512 * hf:512 * hf + 512],
                    gall[:, 128 * nt:128 * nt + 128],
                    adjt[nt][:, 512 * hf:512 * hf + 512],
                    start=(nt == 0), stop=(nt == 7))
        nc.vector.tensor_copy(gxT[:, 0:512], Xg[:, 0:512])
        nc.scalar.copy(gxT[:, 512:1024], Xg[:, 512:1024])

        # ---- persistent ma tiles: [128, 2048] = 4 pairs (t-steps) -------
        # rows 0:6  = even-chunk streams (n 0:512):  x1, m, xg, mg, cg, 1
        # rows 64:70= odd-chunk streams  (n 512:1024)
        ma4 = []
        for i in range(4):
            mai = movap.tile([128, BLK], h16, tag=f"ma{i}", name=f"ma{i}")
            nc.vector.memset(mai[0:64, :], 0.0)
            nc.vector.memset(mai[64:128, :], 0.0)
            nc.sync.dma_start(out=mai[5:6, :], in_=ones_d[:])
            nc.sync.dma_start(out=mai[69:70, :], in_=ones_d[:])
            nc.sync.dma_start(out=mai[4:5, :], in_=cge_d[:])
            nc.sync.dma_start(out=mai[68:69, :], in_=cgo_d[:])
            ma4.append(mai)

        def emit_ma(b):
            mat = ma4[b % 4]
            t0 = 4 * b
            nc.sync.dma_start(out=mat[0:1, :], in_=x1T_t[t0:t0 + 4, 0:512])
            nc.sync.dma_start(out=mat[64:65, :], in_=x1T_t[t0:t0 + 4, 512:1024])
            nc.scalar.dma_start(out=mat[1:2, :], in_=mT_t[t0:t0 + 4, 0:512])
            nc.scalar.dma_start(out=mat[65:66, :],
                                in_=mT_t[t0:t0 + 4, 512:1024])
            nc.gpsimd.dma_start(out=mat[2:3, :], in_=gxT[t0:t0 + 4, 0:512])
            nc.sync.dma_start(out=mat[66:67, :], in_=gxT[t0:t0 + 4, 512:1024])
            nc.gpsimd.dma_start(out=mat[3:4, :],
                                in_=gxT[64 + t0:64 + t0 + 4, 0:512])
            nc.scalar.dma_start(out=mat[67:68, :],
                                in_=gxT[64 + t0:64 + t0 + 4, 512:1024])

        emit_ma(0)
        emit_ma(1)
        emit_ma(2)

        # ---- streaming pipeline -----------------------------------------
        Xs = {}
        tts = {}
        tt_all = {}
        rts = {}
        gcur = {}

        def pe_dep(ap):
            # dependency-carrying PE no-op: pins PE program order to the
            # producer of `ap` so the scheduler cannot hoist the next MMs
            bi = nc.tensor.nop(hint="dep")
            bi.ins.ins = [nc.tensor.lower_ap(ap)]

        def st_A(g):
            X = Xp.tile([128, 1024], f32, tag="X", name=f"X{g}")
            for h in range(2):
                p = 2 * g + h
                mat = ma4[(p // 4) % 4]
                mc = (p % 4) * CHUNK
                nc.tensor.matmul(
                    X[:, 512 * h:512 * h + 512], sa_t[:],
                    mat[:, mc:mc + CHUNK], start=True, stop=True)
            Xs[g] = X

        def st_t(g):
            tt = ttp.tile([128, 1024], h16, tag="tt", name=f"tt{g}")
            X = Xs[g]
            nc.scalar.activation(tt[:, 0:512], X[:, 0:512], AF.Prelu,
                                 bias=0.0, scale=1.0, alpha=a_slope)
            nc.scalar.activation(tt[:, 512:1024], X[:, 512:1024], AF.Prelu,
                                 bias=0.0, scale=1.0, alpha=a_slope)
            tts[g] = tt
            tt_all[g] = tt

        def st_B(g):
            X, tt = Xs[g], tts[g]
            for h in range(2):
                nc.tensor.matmul(
                    X[:, 512 * h:512 * h + 512], sb_t[:],
                    tt[:, 512 * h:512 * h + 512], start=True, stop=True)
            del tts[g]

        def st_rr(g):
            rt = rtp.tile([128, 1024], h16, tag="rt", name=f"rt{g}")
            nc.vector.tensor_scalar(
                out=rt[:], in0=Xs[g][:],
                scalar1=bk2_t[:, 0:1], scalar2=0.0,
                op0=ALU.add, op1=ALU.max)
            rts[g] = rt
            del Xs[g]

        def st_C(g):
            rt = rts[g]
            for h in range(2):
                p = 2 * g + h
                s = p // NSC
                j = p % NSC
                if j == 0:
                    gcur[s] = gp.tile([128, 512], f32, tag="go",
                                      name=f"go{s}")
                gam = gcur[s]
                nc.tensor.matmul(
                    gam[:], sc_t[:, 128 * j:128 * j + 128],
                    rt[:, 512 * h:512 * h + 512],
                    start=(j == 0), stop=(j == NSC - 1),
                    skip_group_check=True)
                if j == NSC - 1:
                    osb = osp.tile([128, 512], f32, tag="os", name=f"os{s}")
                    nc.scalar.copy(osb[0:64, :], gam[0:64, :])
                    nc.sync.dma_start(out=out_d[64 * s:64 * s + 64, :],
                                      in_=osb[0:64, :])
                    del gcur[s]
            del rts[g]

        # lags: B at g-2, C at g-4 -> every PE dep has ~2 iterations of
        # slack against the ACT/DVE elementwise ops.
        for g in range(NGRP + 4):
            if 2 <= g < NGRP + 2:
                st_B(g - 2)
                st_rr(g - 2)
            if g >= 4:
                st_C(g - 4)
            if g < NGRP:
                if g % 2 == 1 and g // 2 + 3 < NBLK:
                    emit_ma(g // 2 + 3)
                st_A(g)
                st_t(g)
    nc.compile()
    return nc


def _get_program(a_slope, bfs_val):
    if "prog" not in _CACHE:
        _CACHE["prog"] = _build_program(a_slope, bfs_val)
    return _CACHE["prog"]


def _make_in_maps(x, mask_f, adj16, folded):
    shared = dict(adj=adj16, cge=folded["cge"], cgo=folded["cgo"],
                  ones=folded["ones"],
                  sa=folded["sa"], sb=folded["sb"], sc=folded["sc"],
                  bk2=folded["bk2"])
    in_maps = []
    for b in range(B):
        xb = np.ascontiguousarray(x[b, 0]).astype(np.float16)   # [N, T]
        mb = np.ascontiguousarray(mask_f[b, 0])                 # [N, T] f16
        m = dict(shared)
        m["xT"] = np.ascontiguousarray(xb.T)
        m["mT"] = np.ascontiguousarray(mb.T)
        m["xn"] = np.ascontiguousarray(
            xb.reshape(8, 128, T).transpose(1, 0, 2).reshape(128, 512))
        m["mn"] = np.ascontiguousarray(
            mb.reshape(8, 128, T).transpose(1, 0, 2).reshape(128, 512))
        in_maps.append(m)
    return in_maps


def kernel(x, mask, W_fs, b_fs, W_in, b_in, adj, W_gc, b_gc, W_lo, b_lo,
           prelu_a, W_ro, b_ro, W_o1, b_o1, W_o2, b_o2):
    x = np.asarray(x, np.float32)
    mask_f = np.asarray(mask, np.float16)
    adj = np.asarray(adj, np.float32)

    folded = _fold_weights(np.asarray(W_fs), np.asarray(b_fs),
                           np.asarray(W_in), np.asarray(b_in),
                           np.asarray(W_gc), np.asarray(b_gc),
                           np.asarray(W_lo), np.asarray(b_lo),
                           float(prelu_a),
                           np.asarray(W_ro), np.asarray(b_ro),
                           np.asarray(W_o1), np.asarray(b_o1),
                           np.asarray(W_o2), np.asarray(b_o2), adj)

    nc = _get_program(folded["a"], folded["bfs"])
    in_maps = _make_in_maps(x, mask_f, adj.astype(np.float16), folded)

    from concourse.bass_utils import run_bass_kernel_spmd
    res = run_bass_kernel_spmd(nc, in_maps, list(range(B)))

    out = np.empty((B, C, N, T), np.float32)
    for b in range(B):
        # out row r = chunk r; chunk 2p+h = (t=p, n-half h)
        ob = np.asarray(res.results[b]["out"]).reshape(T, 2, 512)
        ob = ob.transpose(0, 1, 2).reshape(T, N)   # [t, n]
        out[b, 0] = ob.T + folded["b_o2"]
    return out


# revision 28
# speedup vs baseline: 1.0628x; 1.0104x over previous
"""Bass/Trainium2 kernel for nn_BiGRIL (gnn_message_passing).

Algebra (h == 0, C == 1 make the network collapse):
  x1  = where(mask, x, b_fs)
  v   = PA^T . streams            streams = [x1, m, xg, mg, cg, 1]
        with xg = A^T x1, mg = A^T m, cg = A^T 1   (graph diffusion)
  o   = PReLU(v) = max(a*v, v)    (ACT Prelu, one op per 4 chunks)
  f   = PB^T o                    PB = outer(w_ro1, W_o1[:,0])  (rank-1)
  rr  = relu(f + bk)              bk = W_o1[:,0]*b_ro + b_o1    (DVE, one op)
  out = W_o2 . rr  (+ b_o2 added on host)

Layout: t-major columns, col = t*N + n.  Sharding: data-parallel over
batch (B=8 -> 8 cores), weights + adj replicated, no collectives.

PE mapping: every matmul is full-array K=128/M=128/N=512 (no array
tiling, no mode switches -> the HAM clock gate stays at 8/8 = 2.4 GHz).
Each pass packs TWO 512-col chunks block-diagonally into K:
  A: lhsT rows 0:6  -> cols 0:64  (even chunk streams -> v)
     lhsT rows 64:70-> cols 64:128 (odd chunk streams -> v)
  B: [PB 0; 0 PB], C: col 2j+s = W_o2 on rows 64s:64s+64
so each of A/B/C streams only NT/2 columns.  C accumulates 32 pairs
(64 chunks) of output rows into one PSUM bank via column-shifted
stationaries (start=False); 2 evacuations per kernel.
G-phase keeps adj as the *moving* operand (8 LDWEIGHTS total).
"""

import numpy as np
import sys

sys.path.insert(0, "/opt/trn_rl_repo")

B, C, N, T = 8, 1, 1024, 64
H = 64
NT = N * T            # 65536 columns per core, col = t*N + n
CHUNK = 512
NPAIR = 64            # pair p = t-step p: chunks (2p, 2p+1) = n-halves
NGRP = 32             # grp g = pairs (2g, 2g+1) -> one [128,1024] X tile
BLK = 2048            # ma-tile columns = 4 pairs (4 t-steps)
NBLK = 16
NSC = 32              # column-shifted W_o2 variants (pairs per out bank)

_CACHE = {}


def _fold_weights(W_fs, b_fs, W_in, b_in, W_gc, b_gc, W_lo, b_lo, prelu_a,
                  W_ro, b_ro, W_o1, b_o1, W_o2, b_o2, adj):
    f8 = np.float64
    W_in, b_in = W_in.astype(f8), b_in.astype(f8)
    W_gc, b_gc = W_gc.astype(f8), b_gc.astype(f8)
    W_lo, b_lo = W_lo.astype(f8), b_lo.astype(f8)
    W_ro, b_ro = W_ro.astype(f8), b_ro.astype(f8)
    W_o1, b_o1 = W_o1.astype(f8), b_o1.astype(f8)
    W_o2, b_o2 = W_o2.astype(f8), b_o2.astype(f8)

    W0 = W_in[:, 0]
    W1 = W_in[:, 1]
    Wlo1 = W_lo[:, :H]
    M1 = Wlo1 @ W_gc[:, :H]
    M2 = Wlo1 @ W_gc[:, H:]
    b_fold = Wlo1 @ b_gc + b_lo

    PA6 = np.stack([
        M1 @ W0, M1 @ W1, M2 @ W0, M2 @ W1,
        M2 @ b_in, M1 @ b_in + b_fold,
    ])                                      # [6, 64]

    w_ro1 = W_ro[0, :H]
    W_o1c = W_o1[:, 0]
    PB = np.outer(w_ro1, W_o1c)             # [64(K=o), 64(M=f)]
    bk = W_o1c * b_ro[0] + b_o1             # [64]

    h16, fp = np.float16, np.float32
    sa = np.zeros((128, 128))
    sa[0:6, 0:64] = PA6
    sa[64:70, 64:128] = PA6
    sb = np.zeros((128, 128))
    sb[0:64, 0:64] = PB
    sb[64:128, 64:128] = PB
    sc = np.zeros((128, NSC * 128))
    for j in range(NSC):
        sc[0:64, 128 * j + 2 * j] = W_o2[0]
        sc[64:128, 128 * j + 2 * j + 1] = W_o2[0]
    bk2 = np.zeros((128, 1))
    bk2[0:64, 0] = bk
    bk2[64:128, 0] = bk

    cg = adj.astype(f8).sum(axis=0)         # [N] col sums of adj
    cge = np.tile(cg[:512], 4)              # even-chunk cg row per ma block
    cgo = np.tile(cg[512:], 4)

    return dict(
        sa=sa.astype(h16), sb=sb.astype(h16), sc=sc.astype(h16),
        bk2=bk2.astype(fp), cge=cge.astype(h16), cgo=cgo.astype(h16),
        ones=np.ones(BLK, h16),
        bfs=float(b_fs[0]), a=float(prelu_a), b_o2=float(b_o2[0]),
    )


def _build_program(a_slope, bfs_val):
    import concourse.bass as bass
    import concourse.bacc as bacc
    import concourse.mybir as mybir
    import concourse.tile as tile

    dt = mybir.dt
    f32 = dt.float32
    h16 = dt.float16
    AF = mybir.ActivationFunctionType
    ALU = mybir.AluOpType

    nc = bacc.Bacc("TRN2", target_bir_lowering=False, debug=False,
                   num_devices=B)

    xT_d = nc.dram_tensor("xT", [T, N], h16, kind="ExternalInput")
    mT_d = nc.dram_tensor("mT", [T, N], h16, kind="ExternalInput")
    xn_d = nc.dram_tensor("xn", [128, 512], h16, kind="ExternalInput")
    mn_d = nc.dram_tensor("mn", [128, 512], h16, kind="ExternalInput")
    adj_d = nc.dram_tensor("adj", [N, N], h16, kind="ExternalInput")
    cge_d = nc.dram_tensor("cge", [BLK], h16, kind="ExternalInput")
    ones_d = nc.dram_tensor("ones", [BLK], h16, kind="ExternalInput")
    cgo_d = nc.dram_tensor("cgo", [BLK], h16, kind="ExternalInput")
    sa_d = nc.dram_tensor("sa", [128, 128], h16, kind="ExternalInput")
    sb_d = nc.dram_tensor("sb", [128, 128], h16, kind="ExternalInput")
    sc_d = nc.dram_tensor("sc", [128, NSC * 128], h16, kind="ExternalInput")
    bk2_d = nc.dram_tensor("bk2", [128, 1], f32, kind="ExternalInput")
    out_d = nc.dram_tensor("out", [2 * NPAIR, CHUNK], f32,
                           kind="ExternalOutput")

    from contextlib import ExitStack
    with tile.TileContext(nc) as tc, ExitStack() as ctx:
        const = ctx.enter_context(tc.tile_pool(name="const", bufs=1))
        adjp = ctx.enter_context(tc.tile_pool(name="adjp", bufs=1))
        movap = ctx.enter_context(tc.tile_pool(name="movap", bufs=1))
        ttp = ctx.enter_context(tc.tile_pool(name="ttp", bufs=4))
        rtp = ctx.enter_context(tc.tile_pool(name="rtp", bufs=4))
        osp = ctx.enter_context(tc.tile_pool(name="osp", bufs=2))
        Xp = ctx.enter_context(tc.tile_pool(name="Xp", bufs=3, space="PSUM"))
        gp = ctx.enter_context(tc.tile_pool(name="gp", bufs=2, space="PSUM"))

        sa_t = const.tile([128, 128], h16)
        sb_t = const.tile([128, 128], h16)
        sc_t = const.tile([128, NSC * 128], h16)
        bk2_t = const.tile([128, 1], f32)
        nc.sync.dma_start(out=sa_t[:], in_=sa_d[:])
        nc.sync.dma_start(out=sb_t[:], in_=sb_d[:])
        nc.sync.dma_start(out=sc_t[:], in_=sc_d[:])
        nc.sync.dma_start(out=bk2_t[:], in_=bk2_d[:])

        xT_t = const.tile([T, N], h16)
        mT_t = const.tile([T, N], h16)
        x1T_t = const.tile([T, N], h16)
        xn_t = const.tile([128, 512], h16)
        mn_t = const.tile([128, 512], h16)
        x1n_t = const.tile([128, 512], h16)
        gall = const.tile([128, 1024], h16)
        gxT = const.tile([128, 1024], h16)
        nc.scalar.dma_start(out=xT_t[:], in_=xT_d[:])
        nc.scalar.dma_start(out=mT_t[:], in_=mT_d[:])
        nc.scalar.dma_start(out=xn_t[:], in_=xn_d[:])
        nc.scalar.dma_start(out=mn_t[:], in_=mn_d[:])

        # x1 = (x - bfs)*m + bfs  in both orientations
        nc.vector.scalar_tensor_tensor(
            out=x1T_t[:], in0=xT_t[:], scalar=bfs_val, in1=mT_t[:],
            op0=ALU.subtract, op1=ALU.mult)
        nc.vector.tensor_scalar_add(x1T_t[:], x1T_t[:], bfs_val)
        nc.vector.scalar_tensor_tensor(
            out=x1n_t[:], in0=xn_t[:], scalar=bfs_val, in1=mn_t[:],
            op0=ALU.subtract, op1=ALU.mult)
        nc.vector.tensor_scalar_add(x1n_t[:], x1n_t[:], bfs_val)

        # gall: G stationary [128 n-parts, (x1 64t | m 64t)] per node-group
        for nt in range(8):
            nc.scalar.dma_start(out=gall[:, 128 * nt:128 * nt + 64],
                                in_=x1n_t[:, 64 * nt:64 * nt + 64])
            nc.gpsimd.dma_start(out=gall[:, 128 * nt + 64:128 * nt + 128],
                                in_=mn_t[:, 64 * nt:64 * nt + 64])

        adjt = []
        for nt in range(8):
            at = adjp.tile([128, 1024], h16, tag=f"adjt{nt}",
                           name=f"adjt{nt}")
            nc.sync.dma_start(out=at[:], in_=adj_d[nt * 128:(nt + 1) * 128, :])
            adjt.append(at)

        # ---- G: Xg[t | 64+t, m] = sum_n [x1|m][n,t] * adj[n,m] ----------
        Xg = Xp.tile([128, 1024], f32, tag="X", name="Xg")
        for nt in range(8):
            for hf in range(2):
                nc.tensor.matmul(
                    Xg[:, 512 * hf:512 * hf + 512],
                    gall[:, 128 * nt:128 * nt + 128],
                    adjt[nt][:, 512 * hf:512 * hf + 512],
                    start=(nt == 0), stop=(nt == 7))
        nc.vector.tensor_copy(gxT[:, 0:512], Xg[:, 0:512])
        nc.scalar.copy(gxT[:, 512:1024], Xg[:, 512:1024])

        # ---- persistent ma tiles: [128, 2048] = 4 pairs (t-steps) -------
        # rows 0:6  = even-chunk streams (n 0:512):  x1, m, xg, mg, cg, 1
        # rows 64:70= odd-chunk streams  (n 512:1024)
        ma4 = []
        for i in range(4):
            mai = movap.tile([128, BLK], h16, tag=f"ma{i}", name=f"ma{i}")
            nc.vector.memset(mai[0:64, :], 0.0)
            nc.vector.memset(mai[64:128, :], 0.0)
            nc.sync.dma_start(out=mai[5:6, :], in_=ones_d[:])
            nc.sync.dma_start(out=mai[69:70, :], in_=ones_d[:])
            nc.sync.dma_start(out=mai[4:5, :], in_=cge_d[:])
            nc.sync.dma_start(out=mai[68:69, :], in_=cgo_d[:])
            ma4.append(mai)

        def emit_ma_xm(b):
            mat = ma4[b % 4]
            t0 = 4 * b
            nc.sync.dma_start(out=mat[0:1, :], in_=x1T_t[t0:t0 + 4, 0:512])
            nc.sync.dma_start(out=mat[64:65, :], in_=x1T_t[t0:t0 + 4, 512:1024])
            nc.scalar.dma_start(out=mat[1:2, :], in_=mT_t[t0:t0 + 4, 0:512])
            nc.scalar.dma_start(out=mat[65:66, :],
                                in_=mT_t[t0:t0 + 4, 512:1024])

        def emit_ma_gx(b):
            mat = ma4[b % 4]
            t0 = 4 * b
            nc.gpsimd.dma_start(out=mat[2:3, :], in_=gxT[t0:t0 + 4, 0:512])
            nc.sync.dma_start(out=mat[66:67, :], in_=gxT[t0:t0 + 4, 512:1024])
            nc.gpsimd.dma_start(out=mat[3:4, :],
                                in_=gxT[64 + t0:64 + t0 + 4, 0:512])
            nc.scalar.dma_start(out=mat[67:68, :],
                                in_=gxT[64 + t0:64 + t0 + 4, 512:1024])

        def emit_ma(b):
            emit_ma_xm(b)
            emit_ma_gx(b)

        emit_ma_xm(0)
        emit_ma_xm(1)
        emit_ma_xm(2)
        emit_ma_gx(0)
        emit_ma_gx(1)
        emit_ma_gx(2)

        # ---- streaming pipeline -----------------------------------------
        Xs = {}
        tts = {}
        tt_all = {}
        rts = {}
        gcur = {}

        def pe_dep(ap):
            # dependency-carrying PE no-op: pins PE program order to the
            # producer of `ap` so the scheduler cannot hoist the next MMs
            bi = nc.tensor.nop(hint="dep")
            bi.ins.ins = [nc.tensor.lower_ap(ap)]

        def st_A(g):
            X = Xp.tile([128, 1024], f32, tag="X", name=f"X{g}")
            for h in range(2):
                p = 2 * g + h
                mat = ma4[(p // 4) % 4]
                mc = (p % 4) * CHUNK
                nc.tensor.matmul(
                    X[:, 512 * h:512 * h + 512], sa_t[:],
                    mat[:, mc:mc + CHUNK], start=True, stop=True)
            Xs[g] = X

        def st_t(g):
            tt = ttp.tile([128, 1024], h16, tag="tt", name=f"tt{g}")
            X = Xs[g]
            nc.scalar.activation(tt[:, 0:512], X[:, 0:512], AF.Prelu,
                                 bias=0.0, scale=1.0, alpha=a_slope)
            nc.scalar.activation(tt[:, 512:1024], X[:, 512:1024], AF.Prelu,
                                 bias=0.0, scale=1.0, alpha=a_slope)
            tts[g] = tt
            tt_all[g] = tt

        def st_B(g):
            X, tt = Xs[g], tts[g]
            for h in range(2):
                nc.tensor.matmul(
                    X[:, 512 * h:512 * h + 512], sb_t[:],
                    tt[:, 512 * h:512 * h + 512], start=True, stop=True)
            del tts[g]

        def st_rr(g):
            rt = rtp.tile([128, 1024], h16, tag="rt", name=f"rt{g}")
            nc.vector.tensor_scalar(
                out=rt[:], in0=Xs[g][:],
                scalar1=bk2_t[:, 0:1], scalar2=0.0,
                op0=ALU.add, op1=ALU.max)
            rts[g] = rt
            del Xs[g]

        def st_C(g):
            rt = rts[g]
            for h in range(2):
                p = 2 * g + h
                s = p // NSC
                j = p % NSC
                if j == 0:
                    gcur[s] = gp.tile([128, 512], f32, tag="go",
                                      name=f"go{s}")
                gam = gcur[s]
                nc.tensor.matmul(
                    gam[:], sc_t[:, 128 * j:128 * j + 128],
                    rt[:, 512 * h:512 * h + 512],
                    start=(j == 0), stop=(j == NSC - 1),
                    skip_group_check=True)
                if j == NSC - 1:
                    osb = osp.tile([128, 512], f32, tag="os", name=f"os{s}")
                    nc.scalar.copy(osb[0:64, :], gam[0:64, :])
                    nc.sync.dma_start(out=out_d[64 * s:64 * s + 64, :],
                                      in_=osb[0:64, :])
                    del gcur[s]
            del rts[g]

        # lags: B at g-2, C at g-4 -> every PE dep has ~2 iterations of
        # slack against the ACT/DVE elementwise ops.
        for g in range(NGRP + 4):
            if 2 <= g < NGRP + 2:
                st_B(g - 2)
                st_rr(g - 2)
            if g >= 4:
                st_C(g - 4)
            if g < NGRP:
                if g % 2 == 1 and g // 2 + 3 < NBLK:
                    emit_ma(g // 2 + 3)
                st_A(g)
                st_t(g)
    nc.compile()
    return nc


def _get_program(a_slope, bfs_val):
    if "prog" not in _CACHE:
        _CACHE["prog"] = _build_program(a_slope, bfs_val)
    return _CACHE["prog"]


def _make_in_maps(x, mask_f, adj16, folded):
    shared = dict(adj=adj16, cge=folded["cge"], cgo=folded["cgo"],
                  ones=folded["ones"],
                  sa=folded["sa"], sb=folded["sb"], sc=folded["sc"],
                  bk2=folded["bk2"])
    in_maps = []
    for b in range(B):
        xb = np.ascontiguousarray(x[b, 0]).astype(np.float16)   # [N, T]
        mb = np.ascontiguousarray(mask_f[b, 0])                 # [N, T] f16
        m = dict(shared)
        m["xT"] = np.ascontiguousarray(xb.T)
        m["mT"] = np.ascontiguousarray(mb.T)
        m["xn"] = np.ascontiguousarray(
            xb.reshape(8, 128, T).transpose(1, 0, 2).reshape(128, 512))
        m["mn"] = np.ascontiguousarray(
            mb.reshape(8, 128, T).transpose(1, 0, 2).reshape(128, 512))
        in_maps.append(m)
    return in_maps


def kernel(x, mask, W_fs, b_fs, W_in, b_in, adj, W_gc, b_gc, W_lo, b_lo,
           prelu_a, W_ro, b_ro, W_o1, b_o1, W_o2, b_o2):
    x = np.asarray(x, np.float32)
    mask_f = np.asarray(mask, np.float16)
    adj = np.asarray(adj, np.float32)

    folded = _fold_weights(np.asarray(W_fs), np.asarray(b_fs),
                           np.asarray(W_in), np.asarray(b_in),
                           np.asarray(W_gc), np.asarray(b_gc),
                           np.asarray(W_lo), np.asarray(b_lo),
                           float(prelu_a),
                           np.asarray(W_ro), np.asarray(b_ro),
                           np.asarray(W_o1), np.asarray(b_o1),
                           np.asarray(W_o2), np.asarray(b_o2), adj)

    nc = _get_program(folded["a"], folded["bfs"])
    in_maps = _make_in_maps(x, mask_f, adj.astype(np.float16), folded)

    from concourse.bass_utils import run_bass_kernel_spmd
    res = run_bass_kernel_spmd(nc, in_maps, list(range(B)))

    out = np.empty((B, C, N, T), np.float32)
    for b in range(B):
        # out row r = chunk r; chunk 2p+h = (t=p, n-half h)
        ob = np.asarray(res.results[b]["out"]).reshape(T, 2, 512)
        ob = ob.transpose(0, 1, 2).reshape(T, N)   # [t, n]
        out[b, 0] = ob.T + folded["b_o2"]
    return out


# revision 30
# speedup vs baseline: 1.0847x; 1.0207x over previous
"""Bass/Trainium2 kernel for nn_BiGRIL (gnn_message_passing).

Algebra (h == 0, C == 1 make the network collapse):
  x1  = where(mask, x, b_fs)
  v   = PA^T . streams            streams = [x1, m, xg, mg, cg, 1]
        with xg = A^T x1, mg = A^T m, cg = A^T 1   (graph diffusion)
  o   = PReLU(v) = max(a*v, v)    (ACT Prelu, one op per 4 chunks)
  f   = PB^T o                    PB = outer(w_ro1, W_o1[:,0])  (rank-1)
  rr  = relu(f + bk)              bk = W_o1[:,0]*b_ro + b_o1    (DVE, one op)
  out = W_o2 . rr  (+ b_o2 added on host)

Layout: t-major columns, col = t*N + n.  Sharding: data-parallel over
batch (B=8 -> 8 cores), weights + adj replicated, no collectives.

PE mapping: every matmul is full-array K=128/M=128/N=512 (no array
tiling, no mode switches -> the HAM clock gate stays at 8/8 = 2.4 GHz).
Each pass packs TWO 512-col chunks block-diagonally into K:
  A: lhsT rows 0:6  -> cols 0:64  (even chunk streams -> v)
     lhsT rows 64:70-> cols 64:128 (odd chunk streams -> v)
  B: [PB 0; 0 PB], C: col 2j+s = W_o2 on rows 64s:64s+64
so each of A/B/C streams only NT/2 columns.  C accumulates 32 pairs
(64 chunks) of output rows into one PSUM bank via column-shifted
stationaries (start=False); 2 evacuations per kernel.
G-phase keeps adj as the *moving* operand (8 LDWEIGHTS total).
"""

import numpy as np
import sys

sys.path.insert(0, "/opt/trn_rl_repo")

B, C, N, T = 8, 1, 1024, 64
H = 64
NT = N * T            # 65536 columns per core, col = t*N + n
CHUNK = 512
NPAIR = 64            # pair p = t-step p: chunks (2p, 2p+1) = n-halves
NGRP = 32             # grp g = pairs (2g, 2g+1) -> one [128,1024] X tile
BLK = 2048            # ma-tile columns = 4 pairs (4 t-steps)
NBLK = 16
NSC = 32              # column-shifted W_o2 variants (pairs per out bank)

_CACHE = {}


def _fold_weights(W_fs, b_fs, W_in, b_in, W_gc, b_gc, W_lo, b_lo, prelu_a,
                  W_ro, b_ro, W_o1, b_o1, W_o2, b_o2, adj):
    f8 = np.float64
    W_in, b_in = W_in.astype(f8), b_in.astype(f8)
    W_gc, b_gc = W_gc.astype(f8), b_gc.astype(f8)
    W_lo, b_lo = W_lo.astype(f8), b_lo.astype(f8)
    W_ro, b_ro = W_ro.astype(f8), b_ro.astype(f8)
    W_o1, b_o1 = W_o1.astype(f8), b_o1.astype(f8)
    W_o2, b_o2 = W_o2.astype(f8), b_o2.astype(f8)

    W0 = W_in[:, 0]
    W1 = W_in[:, 1]
    Wlo1 = W_lo[:, :H]
    M1 = Wlo1 @ W_gc[:, :H]
    M2 = Wlo1 @ W_gc[:, H:]
    b_fold = Wlo1 @ b_gc + b_lo

    PA6 = np.stack([
        M1 @ W0, M1 @ W1, M2 @ W0, M2 @ W1,
        M2 @ b_in, M1 @ b_in + b_fold,
    ])                                      # [6, 64]

    w_ro1 = W_ro[0, :H]
    W_o1c = W_o1[:, 0]
    PB = np.outer(w_ro1, W_o1c)             # [64(K=o), 64(M=f)]
    bk = W_o1c * b_ro[0] + b_o1             # [64]

    h16, fp = np.float16, np.float32
    sa = np.zeros((128, 128))
    sa[0:6, 0:64] = PA6
    sa[64:70, 64:128] = PA6
    sb = np.zeros((128, 128))
    sb[0:64, 0:64] = PB
    sb[64:128, 64:128] = PB
    sc = np.zeros((128, NSC * 128))
    for j in range(NSC):
        sc[0:64, 128 * j + 2 * j] = W_o2[0]
        sc[64:128, 128 * j + 2 * j + 1] = W_o2[0]
    bk2 = np.zeros((128, 1))
    bk2[0:64, 0] = bk
    bk2[64:128, 0] = bk

    cg = adj.astype(f8).sum(axis=0)         # [N] col sums of adj
    mainit = np.zeros((128, BLK))           # static part of every ma tile
    mainit[4, :] = np.tile(cg[:512], 4)     # even-chunk cg row
    mainit[5, :] = 1.0
    mainit[68, :] = np.tile(cg[512:], 4)    # odd-chunk cg row
    mainit[69, :] = 1.0

    return dict(
        sa=sa.astype(h16), sb=sb.astype(h16), sc=sc.astype(h16),
        bk2=bk2.astype(fp), mainit=mainit.astype(h16),
        bfs=float(b_fs[0]), a=float(prelu_a), b_o2=float(b_o2[0]),
    )


def _build_program(a_slope, bfs_val):
    import concourse.bass as bass
    import concourse.bacc as bacc
    import concourse.mybir as mybir
    import concourse.tile as tile

    dt = mybir.dt
    f32 = dt.float32
    h16 = dt.float16
    AF = mybir.ActivationFunctionType
    ALU = mybir.AluOpType

    nc = bacc.Bacc("TRN2", target_bir_lowering=False, debug=False,
                   num_devices=B)

    xT_d = nc.dram_tensor("xT", [T, N], h16, kind="ExternalInput")
    mT_d = nc.dram_tensor("mT", [T, N], h16, kind="ExternalInput")
    xn_d = nc.dram_tensor("xn", [128, 512], h16, kind="ExternalInput")
    mn_d = nc.dram_tensor("mn", [128, 512], h16, kind="ExternalInput")
    adj_d = nc.dram_tensor("adj", [N, N], h16, kind="ExternalInput")
    mainit_d = nc.dram_tensor("mainit", [128, BLK], h16,
                              kind="ExternalInput")
    sa_d = nc.dram_tensor("sa", [128, 128], h16, kind="ExternalInput")
    sb_d = nc.dram_tensor("sb", [128, 128], h16, kind="ExternalInput")
    sc_d = nc.dram_tensor("sc", [128, NSC * 128], h16, kind="ExternalInput")
    bk2_d = nc.dram_tensor("bk2", [128, 1], f32, kind="ExternalInput")
    out_d = nc.dram_tensor("out", [2 * NPAIR, CHUNK], f32,
                           kind="ExternalOutput")

    from contextlib import ExitStack
    with tile.TileContext(nc) as tc, ExitStack() as ctx:
        const = ctx.enter_context(tc.tile_pool(name="const", bufs=1))
        adjp = ctx.enter_context(tc.tile_pool(name="adjp", bufs=1))
        movap = ctx.enter_context(tc.tile_pool(name="movap", bufs=1))
        ttp = ctx.enter_context(tc.tile_pool(name="ttp", bufs=4))
        rtp = ctx.enter_context(tc.tile_pool(name="rtp", bufs=4))
        osp = ctx.enter_context(tc.tile_pool(name="osp", bufs=2))
        Xp = ctx.enter_context(tc.tile_pool(name="Xp", bufs=3, space="PSUM"))
        gp = ctx.enter_context(tc.tile_pool(name="gp", bufs=2, space="PSUM"))

        sa_t = const.tile([128, 128], h16)
        sb_t = const.tile([128, 128], h16)
        sc_t = const.tile([128, NSC * 128], h16)
        bk2_t = const.tile([128, 1], f32)
        nc.sync.dma_start(out=sa_t[:], in_=sa_d[:])
        nc.sync.dma_start(out=sb_t[:], in_=sb_d[:])
        nc.sync.dma_start(out=sc_t[:], in_=sc_d[:])
        nc.sync.dma_start(out=bk2_t[:], in_=bk2_d[:])

        xT_t = const.tile([T, N], h16)
        mT_t = const.tile([T, N], h16)
        x1T_t = const.tile([T, N], h16)
        xn_t = const.tile([128, 512], h16)
        mn_t = const.tile([128, 512], h16)
        x1n_t = const.tile([128, 512], h16)
        gall = const.tile([128, 1024], h16)
        gxT = const.tile([128, 1024], h16)
        nc.scalar.dma_start(out=xT_t[:], in_=xT_d[:])
        nc.scalar.dma_start(out=mT_t[:], in_=mT_d[:])
        nc.scalar.dma_start(out=xn_t[:], in_=xn_d[:])
        nc.scalar.dma_start(out=mn_t[:], in_=mn_d[:])

        # x1 = (x - bfs)*m + bfs  in both orientations
        nc.vector.scalar_tensor_tensor(
            out=x1T_t[:], in0=xT_t[:], scalar=bfs_val, in1=mT_t[:],
            op0=ALU.subtract, op1=ALU.mult)
        nc.vector.tensor_scalar_add(x1T_t[:], x1T_t[:], bfs_val)
        nc.vector.scalar_tensor_tensor(
            out=x1n_t[:], in0=xn_t[:], scalar=bfs_val, in1=mn_t[:],
            op0=ALU.subtract, op1=ALU.mult)
        nc.vector.tensor_scalar_add(x1n_t[:], x1n_t[:], bfs_val)

        # gall: G stationary [128 n-parts, (x1 64t | m 64t)] per node-group
        for nt in range(8):
            nc.scalar.dma_start(out=gall[:, 128 * nt:128 * nt + 64],
                                in_=x1n_t[:, 64 * nt:64 * nt + 64])
            nc.gpsimd.dma_start(out=gall[:, 128 * nt + 64:128 * nt + 128],
                                in_=mn_t[:, 64 * nt:64 * nt + 64])

        adjt = []
        for nt in range(8):
            at = adjp.tile([128, 1024], h16, tag=f"adjt{nt}",
                           name=f"adjt{nt}")
            nc.sync.dma_start(out=at[:], in_=adj_d[nt * 128:(nt + 1) * 128, :])
            adjt.append(at)

        # ---- G: Xg[t | 64+t, m] = sum_n [x1|m][n,t] * adj[n,m] ----------
        Xg = Xp.tile([128, 1024], f32, tag="X", name="Xg")
        for nt in range(8):
            for hf in range(2):
                nc.tensor.matmul(
                    Xg[:, 512 * hf:512 * hf + 512],
                    gall[:, 128 * nt:128 * nt + 128],
                    adjt[nt][:, 512 * hf:512 * hf + 512],
                    start=(nt == 0), stop=(nt == 7))
        nc.vector.tensor_copy(gxT[:, 0:512], Xg[:, 0:512])
        nc.scalar.copy(gxT[:, 512:1024], Xg[:, 512:1024])

        # ---- persistent ma tiles: [128, 2048] = 4 pairs (t-steps) -------
        # rows 0:6  = even-chunk streams (n 0:512):  x1, m, xg, mg, cg, 1
        # rows 64:70= odd-chunk streams  (n 512:1024)
        ma4 = []
        iq = [nc.sync, nc.scalar, nc.gpsimd, nc.sync]
        for i in range(4):
            mai = movap.tile([128, BLK], h16, tag=f"ma{i}", name=f"ma{i}")
            iq[i].dma_start(out=mai[:], in_=mainit_d[:])
            ma4.append(mai)

        def emit_ma_xm(b):
            mat = ma4[b % 4]
            t0 = 4 * b
            nc.sync.dma_start(out=mat[0:1, :], in_=x1T_t[t0:t0 + 4, 0:512])
            nc.sync.dma_start(out=mat[64:65, :], in_=x1T_t[t0:t0 + 4, 512:1024])
            nc.scalar.dma_start(out=mat[1:2, :], in_=mT_t[t0:t0 + 4, 0:512])
            nc.scalar.dma_start(out=mat[65:66, :],
                                in_=mT_t[t0:t0 + 4, 512:1024])

        def emit_ma_gx(b):
            mat = ma4[b % 4]
            t0 = 4 * b
            nc.gpsimd.dma_start(out=mat[2:3, :], in_=gxT[t0:t0 + 4, 0:512])
            nc.sync.dma_start(out=mat[66:67, :], in_=gxT[t0:t0 + 4, 512:1024])
            nc.gpsimd.dma_start(out=mat[3:4, :],
                                in_=gxT[64 + t0:64 + t0 + 4, 0:512])
            nc.scalar.dma_start(out=mat[67:68, :],
                                in_=gxT[64 + t0:64 + t0 + 4, 512:1024])

        def emit_ma(b):
            emit_ma_xm(b)
            emit_ma_gx(b)

        emit_ma_xm(0)
        emit_ma_xm(1)
        emit_ma_xm(2)
        emit_ma_gx(0)
        emit_ma_gx(1)
        emit_ma_gx(2)

        # ---- streaming pipeline -----------------------------------------
        Xs = {}
        tts = {}
        tt_all = {}
        rts = {}
        gcur = {}

        def pe_dep(ap):
            # dependency-carrying PE no-op: pins PE program order to the
            # producer of `ap` so the scheduler cannot hoist the next MMs
            bi = nc.tensor.nop(hint="dep")
            bi.ins.ins = [nc.tensor.lower_ap(ap)]

        def st_A(g):
            X = Xp.tile([128, 1024], f32, tag="X", name=f"X{g}")
            for h in range(2):
                p = 2 * g + h
                mat = ma4[(p // 4) % 4]
                mc = (p % 4) * CHUNK
                nc.tensor.matmul(
                    X[:, 512 * h:512 * h + 512], sa_t[:],
                    mat[:, mc:mc + CHUNK], start=True, stop=True)
            Xs[g] = X

        def st_t(g):
            tt = ttp.tile([128, 1024], h16, tag="tt", name=f"tt{g}")
            X = Xs[g]
            nc.scalar.activation(tt[:, 0:512], X[:, 0:512], AF.Prelu,
                                 bias=0.0, scale=1.0, alpha=a_slope)
            nc.scalar.activation(tt[:, 512:1024], X[:, 512:1024], AF.Prelu,
                                 bias=0.0, scale=1.0, alpha=a_slope)
            tts[g] = tt
            tt_all[g] = tt

        def st_B(g):
            X, tt = Xs[g], tts[g]
            for h in range(2):
                nc.tensor.matmul(
                    X[:, 512 * h:512 * h + 512], sb_t[:],
                    tt[:, 512 * h:512 * h + 512], start=True, stop=True)
            del tts[g]

        def st_rr(g):
            rt = rtp.tile([128, 1024], h16, tag="rt", name=f"rt{g}")
            nc.vector.tensor_scalar(
                out=rt[:], in0=Xs[g][:],
                scalar1=bk2_t[:, 0:1], scalar2=0.0,
                op0=ALU.add, op1=ALU.max)
            rts[g] = rt
            del Xs[g]

        def st_C(g):
            rt = rts[g]
            for h in range(2):
                p = 2 * g + h
                s = p // NSC
                j = p % NSC
                if j == 0:
                    gcur[s] = gp.tile([128, 512], f32, tag="go",
                                      name=f"go{s}")
                gam = gcur[s]
                nc.tensor.matmul(
                    gam[:], sc_t[:, 128 * j:128 * j + 128],
                    rt[:, 512 * h:512 * h + 512],
                    start=(j == 0), stop=(j == NSC - 1),
                    skip_group_check=True)
                if j == NSC - 1:
                    osb = osp.tile([128, 512], f32, tag="os", name=f"os{s}")
                    nc.scalar.copy(osb[0:64, :], gam[0:64, :])
                    nc.sync.dma_start(out=out_d[64 * s:64 * s + 64, :],
                                      in_=osb[0:64, :])
                    del gcur[s]
            del rts[g]

        # lags: B at g-2, C at g-4 -> every PE dep has ~2 iterations of
        # slack against the ACT/DVE elementwise ops.
        for g in range(NGRP + 4):
            if 2 <= g < NGRP + 2:
                st_B(g - 2)
                st_rr(g - 2)
            if g >= 4:
                st_C(g - 4)
            if g < NGRP:
                if g % 2 == 1 and g // 2 + 3 < NBLK:
                    emit_ma(g // 2 + 3)
                st_A(g)
                st_t(g)
    nc.compile()
    return nc


def _get_program(a_slope, bfs_val):
    if "prog" not in _CACHE:
        _CACHE["prog"] = _build_program(a_slope, bfs_val)
    return _CACHE["prog"]


def _make_in_maps(x, mask_f, adj16, folded):
    shared = dict(adj=adj16, mainit=folded["mainit"],
                  sa=folded["sa"], sb=folded["sb"], sc=folded["sc"],
                  bk2=folded["bk2"])
    in_maps = []
    for b in range(B):
        xb = np.ascontiguousarray(x[b, 0]).astype(np.float16)   # [N, T]
        mb = np.ascontiguousarray(mask_f[b, 0])                 # [N, T] f16
        m = dict(shared)
        m["xT"] = np.ascontiguousarray(xb.T)
        m["mT"] = np.ascontiguousarray(mb.T)
        m["xn"] = np.ascontiguousarray(
            xb.reshape(8, 128, T).transpose(1, 0, 2).reshape(128, 512))
        m["mn"] = np.ascontiguousarray(
            mb.reshape(8, 128, T).transpose(1, 0, 2).reshape(128, 512))
        in_maps.append(m)
    return in_maps


def kernel(x, mask, W_fs, b_fs, W_in, b_in, adj, W_gc, b_gc, W_lo, b_lo,
           prelu_a, W_ro, b_ro, W_o1, b_o1, W_o2, b_o2):
    x = np.asarray(x, np.float32)
    mask_f = np.asarray(mask, np.float16)
    adj = np.asarray(adj, np.float32)

    folded = _fold_weights(np.asarray(W_fs), np.asarray(b_fs),
                           np.asarray(W_in), np.asarray(b_in),
                           np.asarray(W_gc), np.asarray(b_gc),
                           np.asarray(W_lo), np.asarray(b_lo),
                           float(prelu_a),
                           np.asarray(W_ro), np.asarray(b_ro),
                           np.asarray(W_o1), np.asarray(b_o1),
                           np.asarray(W_o2), np.asarray(b_o2), adj)

    nc = _get_program(folded["a"], folded["bfs"])
    in_maps = _make_in_maps(x, mask_f, adj.astype(np.float16), folded)

    from concourse.bass_utils import run_bass_kernel_spmd
    res = run_bass_kernel_spmd(nc, in_maps, list(range(B)))

    out = np.empty((B, C, N, T), np.float32)
    for b in range(B):
        # out row r = chunk r; chunk 2p+h = (t=p, n-half h)
        ob = np.asarray(res.results[b]["out"]).reshape(T, 2, 512)
        ob = ob.transpose(0, 1, 2).reshape(T, N)   # [t, n]
        out[b, 0] = ob.T + folded["b_o2"]
    return out
